# revision 5
# baseline (speedup 1.0000x reference)
"""CCRNN Trainium2 kernel: feature MLP + embedding lookup + 40-step LSTM + vocab projection.

Sharding: data-parallel over batch B=256 -> 8 cores x 32. Weights replicated.

Per-core plan (B_loc=32, T=40, IN=2048, E=512, H=1024, V=5000):
  ph1: gather shifted embeddings (indirect DMA, bf16) + PE-transpose -> embT [E, tb]
  ph2: features = X @ W_f.T + b_f (bf16 matmul), augment with ones col, transpose,
       replicate 4x along M -> featT_rep [P, 5, 128] bf16
  ph3: gx[t,b,:] stored in SBUF as gxsb [128(4t',32b), MT, 8n, 512] bf16.
       Per (n, m): 4 emb matmuls into psum; vector add with fg_sb (feature+bias
       part, computed once per n on replicated featT) -> gxsb slice.
  ph4: LSTM recurrence, bf16 matmuls col-group packed (tile_position=(0,32q)).
       gt [128(q,b), 1024] prefetched per-step from gxsb via 8 SBUF->SBUF DMAs
       (partition shuffle (t',b)->(q,b)). h PE-transposed into hsT.
  ph5: logits = hs @ W_out.T + b_out (bf16, M=128 tiles), wout preloaded with
       contiguous 10KB/partition lines.

Gate column permutation (newcol -> orig), n = a*4+q in 0..7, j in 0..511:
  a=0: [i_q | f_q], a=1: [g_q | o_q], where gate rows (torch order) i,f,g,o
  and quarter q covers H rows 256q..256q+255.
"""

import os
import sys
import types
import contextlib

import numpy as np
import ml_dtypes

# ---- environment bootstrap (works in a bare dir; paths are machine-level) ----
for _p in ("/root/.axon_site", "/root/.axon_site/_ro/trn_rl_repo",
           "/root/.axon_site/_ro/pypackages"):
    if _p not in sys.path and os.path.isdir(_p):
        sys.path.insert(0, _p)

# shim the NTFF profile hook if the image's antenv lacks it (needed for trace=True)
try:
    from antenv.axon_hooks import get_axon_ntff_profile_hook  # noqa: F401
except ImportError:
    try:
        from trn_agent_boot.trn_boot import _ntff_profile_via_ctypes
        _m = types.ModuleType("antenv.axon_hooks")
        _hook = _ntff_profile_via_ctypes("/opt/axon/libaxon_pjrt.so")
        _m.get_axon_ntff_profile_hook = lambda: _hook
        _m.set_axon_ntff_profile_hook = lambda h: None
        import antenv  # noqa: F401
        sys.modules["antenv.axon_hooks"] = _m
    except Exception:
        pass

import concourse.bass as bass
import concourse.tile as tile
from concourse import mybir, bacc
from concourse import bass_utils
bass_utils.upload_artifacts = lambda tmpdir: f"file://{tmpdir}"
from concourse.bass import ds, ts
from concourse.bass_utils import run_bass_kernel_spmd
from concourse.masks import make_identity

P = 128
B, T = 256, 40
IN, E, H, V = 2048, 512, 1024, 5000
G4 = 4 * H
NCORES = 8
BL = B // NCORES            # 32 per core
TB = T * BL                 # 1280 (t*32+b) rows
MT = TB // P                # 10 M-tiles
F32 = mybir.dt.float32
F32R = mybir.dt.float32r
BF16 = mybir.dt.bfloat16
I32 = mybir.dt.int32

NV = 500                    # vocab chunk (psum free)
NVC = V // NV               # 10


def gate_perm() -> np.ndarray:
    """newcol -> orig index over the 4096 gate dim.

    bank0 (first psum bank) = [f | g] (the LSTM critical-path gates),
    bank1 = [i | o]; torch gate order is i,f,g,o.
    """
    perm = np.empty(G4, dtype=np.int64)
    for n in range(8):
        a, q = divmod(n, 4)
        for j in range(512):
            if a == 0:
                g = 1 if j < 256 else 2                # f | g
            else:
                g = 0 if j < 256 else 3                # i | o
            jj = j if j < 256 else j - 256
            perm[n * 512 + j] = g * H + q * 256 + jj
    return perm


def build_nc(debug: bool = False):
    nc = bacc.Bacc("TRN2", target_bir_lowering=False, debug=False)

    xt = nc.declare_dram_parameter("xt", [IN, BL], BF16, isOutput=False)
    lab = nc.declare_dram_parameter("lab", [TB, 1], I32, isOutput=False)
    wft = nc.declare_dram_parameter("wft", [IN, E], BF16, isOutput=False)
    bf = nc.declare_dram_parameter("bf", [1, E], F32, isOutput=False)
    embt = nc.declare_dram_parameter("embt", [V + 1, E], BF16, isOutput=False)
    wihe = nc.declare_dram_parameter("wihe", [E, G4], BF16, isOutput=False)
    wihf = nc.declare_dram_parameter("wihf", [E + P, G4], BF16, isOutput=False)
    whh = nc.declare_dram_parameter("whh", [H, G4], BF16, isOutput=False)
    wout = nc.declare_dram_parameter("wout", [H, V], BF16, isOutput=False)
    bout = nc.declare_dram_parameter("bout", [1, V], F32, isOutput=False)
    logits = nc.declare_dram_parameter("logits", [NVC, MT, P, NV], F32, isOutput=True)
    if debug:
        d_feat = nc.declare_dram_parameter("d_feat", [BL, E], F32, isOutput=True)
        d_gx0 = nc.declare_dram_parameter("d_gx0", [P, 1024], F32, isOutput=True)
        d_g0 = nc.declare_dram_parameter("d_g0", [P, 1024], F32, isOutput=True)
        d_h1 = nc.declare_dram_parameter("d_h1", [P, 256], F32, isOutput=True)
        d_c1 = nc.declare_dram_parameter("d_c1", [P, 256], F32, isOutput=True)
        d_g1 = nc.declare_dram_parameter("d_g1", [P, 1024], F32, isOutput=True)
        d_h2 = nc.declare_dram_parameter("d_h2", [P, 256], F32, isOutput=True)
        d_hsT = nc.declare_dram_parameter("d_hsT", [P, 8, BL], BF16, isOutput=True)

    Sig = mybir.ActivationFunctionType.Sigmoid
    Tanh = mybir.ActivationFunctionType.Tanh

    with tile.TileContext(nc) as tc, contextlib.ExitStack() as top:
        pc = top.enter_context(tc.tile_pool(name="pc", bufs=1))
        identb = pc.tile([P, P], BF16, tag="identb")
        make_identity(nc, identb[:])
        identf = pc.tile([P, P], F32, tag="identf")
        make_identity(nc, identf[:])

        # persistent across ph4+ph5
        prec = top.enter_context(tc.tile_pool(name="prec", bufs=1))
        hsT = prec.tile([P, 8, T + 1, BL], BF16, tag="hsT")
        cst = prec.tile([P, 256], F32, tag="cst")

        with tc.tile_pool(name="pgx", bufs=1) as pgx:
            # gx, bf16, natural ph3 layout: [128(4t',32b), m, n, 512]
            gxsb = pgx.tile([P, MT, 8, 512], BF16, tag="gxsb")

            with tc.tile_pool(name="pembT", bufs=1) as pembT, \
                 tc.tile_pool(name="pfeat", bufs=1) as pfeat:
                embT = pembT.tile([P, E // P, TB], BF16, tag="embT")
                featT_rep = pfeat.tile([P, 5, P], BF16, tag="featT_rep")

                # ---------------- phase 1: gather + transpose embeddings ----------------
                with (
                    tc.tile_pool(name="pgath", bufs=1) as pgath,
                    tc.tile_pool(name="ps1", bufs=4, space="PSUM") as ps1,
                ):
                    lab_sb = pgath.tile([P, MT, 1], I32, tag="lab")
                    nc.sync.dma_start(lab_sb[:], lab.rearrange("(m p) o -> p m o", p=P))
                    gath = pgath.tile([P, MT, E], BF16, tag="gath")
                    for m in range(MT):
                        nc.gpsimd.indirect_dma_start(
                            out=gath[:, m, :], out_offset=None, in_=embt[:],
                            in_offset=bass.IndirectOffsetOnAxis(ap=lab_sb[:, m, :], axis=0),
                        )
                    for m in range(MT):
                        for k in range(E // P):
                            pst = ps1.tile([P, P], BF16, space="PSUM", tag="pst")
                            nc.tensor.transpose(pst[:], gath[:, m, ds(k * P, P)], identb[:])
                            nc.vector.tensor_copy(embT[:, k, ds(m * P, P)], pst[:])

                # ---------------- phase 2: features ----------------
                with (
                    tc.tile_pool(name="pph2", bufs=1) as p2,
                    tc.tile_pool(name="ps2", bufs=2, space="PSUM") as ps2,
                    tc.tile_pool(name="ps2f", bufs=1, space="PSUM") as ps2f,
                ):
                    xt_sb = p2.tile([P, IN // P, BL], BF16, tag="xt")
                    nc.sync.dma_start(xt_sb[:], xt.rearrange("(k p) b -> p k b", p=P))
                    wft_sb = p2.tile([P, IN // P, E], BF16, tag="wft")
                    nc.sync.dma_start(wft_sb[:], wft.rearrange("(k p) e -> p k e", p=P))
                    bf_sb = p2.tile([BL, E], F32, tag="bf")
                    nc.sync.dma_start(bf_sb[:], bf[0][None, :].broadcast_to((BL, E)))

                    psf = ps2f.tile([BL, E], F32, space="PSUM", tag="psf")
                    nk = IN // P
                    for k in range(nk):
                        nc.tensor.matmul(psf[:], xt_sb[:, k, :], wft_sb[:, k, :],
                                         start=(k == 0), stop=(k == nk - 1))
                    fp = p2.tile([P, E + P], F32, tag="fp")
                    nc.gpsimd.memset(fp[:], 0.0)
                    nc.vector.tensor_add(fp[:BL, 0:E], psf[:], bf_sb[:])
                    nc.gpsimd.memset(fp[:BL, E:E + 1], 1.0)
                    if debug:
                        nc.sync.dma_start(d_feat[:], fp[:BL, 0:E])

                    for kk in range(5):
                        pst2 = ps2.tile([P, P], F32, space="PSUM", tag="pst2")
                        nc.tensor.transpose(pst2[:], fp[:, ds(kk * P, P)], identf[:])
                        for r in range(4):
                            nc.vector.tensor_copy(featT_rep[:, kk, ds(r * BL, BL)],
                                                  pst2[:, 0:BL])

                # ---------------- phase 3: gx ----------------
                with (
                    tc.tile_pool(name="p3w", bufs=2) as p3w,
                    tc.tile_pool(name="ps3", bufs=3, space="PSUM") as ps3,
                    tc.tile_pool(name="ps3f", bufs=2, space="PSUM") as ps3f,
                ):
                    wihe3 = wihe.rearrange("(k p) c -> p k c", p=P)
                    wihf3 = wihf.rearrange("(k p) c -> p k c", p=P)
                    for n in range(8):
                        wihe_n = p3w.tile([P, E // P, 512], BF16, tag="wihe")
                        nc.sync.dma_start(wihe_n[:], wihe3[:, :, ds(n * 512, 512)])
                        wihf_n = p3w.tile([P, 5, 512], BF16, tag="wihf")
                        nc.sync.dma_start(wihf_n[:], wihf3[:, :, ds(n * 512, 512)])

                        psfg = ps3f.tile([P, 512], F32, space="PSUM", tag="psfg")
                        for kk in range(5):
                            nc.tensor.matmul(psfg[:], featT_rep[:, kk, :], wihf_n[:, kk, :],
                                             start=(kk == 0), stop=(kk == 4))
                        fg_sb = p3w.tile([P, 512], BF16, tag="fg")
                        nc.scalar.copy(fg_sb[:], psfg[:])

                        for m in range(MT):
                            ps = ps3.tile([P, 512], F32, space="PSUM", tag="ps3")
                            for k in range(E // P):
                                nc.tensor.matmul(ps[:], embT[:, k, ds(m * P, P)],
                                                 wihe_n[:, k, :],
                                                 start=(k == 0), stop=(k == E // P - 1))
                            nc.vector.tensor_add(gxsb[:, m, n, :], ps[:], fg_sb[:])

            # ---------------- phase 4: recurrence ----------------
            with (
                tc.tile_pool(name="pwhh", bufs=1) as pwhh,
                tc.tile_pool(name="p4", bufs=2) as p4,
                tc.tile_pool(name="p4g", bufs=1) as p4g,
                tc.tile_pool(name="ps4", bufs=3, space="PSUM") as ps4,
                tc.tile_pool(name="ps4t", bufs=2, space="PSUM") as ps4t,
            ):
                whh_sb = pwhh.tile([P, H // P, G4], BF16, tag="whh")
                whh3 = whh.rearrange("(k p) c -> p k c", p=P)
                for k in range(H // P):
                    nc.sync.dma_start(whh_sb[:, k, :], whh3[:, k, :])
                nc.gpsimd.memset(hsT[:, :, 0, :], 0.0)
                nc.gpsimd.memset(cst[:], 0.0)
                NGT = 3
                gxt = [p4g.tile([P, 1024], BF16, tag=f"gxt{i}", name=f"gxt{i}")
                       for i in range(NGT)]
                tc.strict_bb_all_engine_barrier()

                def prefetch_gt(t):
                    # gt[(q,b), a*512+j] <- gxsb[(t%4, b), t//4, a*4+q, j]
                    gt = gxt[t % NGT]
                    tp, m = t % 4, t // 4
                    for n8 in range(8):
                        a8, q8 = divmod(n8, 4)
                        eng = nc.sync if n8 < 4 else nc.gpsimd
                        eng.dma_start(
                            gt[ds(32 * q8, 32), ds(512 * a8, 512)],
                            gxsb[ds(32 * tp, 32), m, n8, :])
                    return gt

                gts = {}
                for t in range(2):
                    gts[t] = prefetch_gt(t)

                hsT_v = hsT[:].rearrange("p (q h) t b -> p q h t b", h=2)

                for t in range(T):
                    if t + 2 < T:
                        gts[t + 2] = prefetch_gt(t + 2)
                    gt = gts.pop(t)
                    pss = []
                    for a in range(2):
                        ps = ps4.tile([P, 512], F32, space="PSUM", tag="ps4")
                        pss.append(ps)
                        for k in range(H // P):
                            for q in range(4):
                                nc.tensor.matmul(
                                    ps[ds(32 * q, 32), :],
                                    hsT[:, k, t, :],
                                    whh_sb[:, k, ds((a * 4 + q) * 512, 512)],
                                    start=(k == 0), stop=(k == H // P - 1),
                                    tile_position=(0, 32 * q))
                    # bank0 = [f | g], bank1 = [i | o]
                    ga = p4.tile([P, 512], F32, tag="ga")
                    gb = p4.tile([P, 512], F32, tag="gb")
                    nc.vector.tensor_add(ga[:, 0:256], pss[0][:, 0:256], gt[:, 0:256])
                    nc.vector.tensor_add(ga[:, 256:512], pss[0][:, 256:512],
                                         gt[:, 256:512])
                    sf = p4.tile([P, 256], F32, tag="sf")
                    tg = p4.tile([P, 256], F32, tag="tg")
                    nc.scalar.activation(sf[:], ga[:, 0:256], Sig)
                    nc.scalar.activation(tg[:], ga[:, 256:512], Tanh)
                    nc.gpsimd.tensor_mul(cst[:], sf[:], cst[:])
                    nc.vector.tensor_add(gb[:, 0:256], pss[1][:, 0:256], gt[:, 512:768])
                    si = p4.tile([P, 256], F32, tag="si")
                    nc.scalar.activation(si[:], gb[:, 0:256], Sig)
                    nc.vector.tensor_add(gb[:, 256:512], pss[1][:, 256:512],
                                         gt[:, 768:1024])
                    so = p4.tile([P, 256], F32, tag="so")
                    nc.scalar.activation(so[:], gb[:, 256:512], Sig)
                    t1 = p4.tile([P, 256], F32, tag="t1")
                    nc.gpsimd.tensor_mul(t1[:], si[:], tg[:])
                    nc.gpsimd.tensor_add(cst[:], cst[:], t1[:])
                    tc_ = p4.tile([P, 256], F32, tag="tc")
                    nc.scalar.activation(tc_[:], cst[:], Tanh)
                    h128 = p4.tile([P, 256], F32, tag="h128")
                    nc.vector.tensor_mul(h128[:], so[:], tc_[:])
                    if debug and t == 0:
                        nc.sync.dma_start(d_g0[:, 0:512], ga[:])
                        nc.sync.dma_start(d_g0[:, 512:1024], gb[:])
                        nc.sync.dma_start(d_gx0[:, 0:512], gt[:, 0:512])
                        nc.sync.dma_start(d_gx0[:, 512:1024], gt[:, 512:1024])
                    if debug and t == 0:
                        nc.sync.dma_start(d_h1[:], h128[:])
                        nc.sync.dma_start(d_c1[:], cst[:])
                    if debug and t == 1:
                        nc.sync.dma_start(d_hsT[:], hsT[:, :, 1, :])
                        nc.sync.dma_start(d_g1[:, 0:512], ga[:])
                        nc.sync.dma_start(d_g1[:, 512:1024], gb[:])
                        nc.sync.dma_start(d_h2[:], h128[:])

                    for half in range(2):
                        pst4 = ps4t.tile([P, P], F32, space="PSUM", tag="pst4")
                        nc.tensor.transpose(pst4[:], h128[:, ds(half * P, P)], identf[:])
                        nc.scalar.copy(
                            hsT_v[:, :, half, t + 1, :],
                            pst4[:].rearrange("p (q b) -> p q b", b=BL))

        # ---------------- phase 5: output projection ----------------
        with (
            tc.tile_pool(name="p5w", bufs=1) as p5w,
            tc.tile_pool(name="p5o", bufs=3) as p5o,
            tc.tile_pool(name="p5b", bufs=1) as p5b,
            tc.tile_pool(name="ps5", bufs=4, space="PSUM") as ps5,
        ):
            tc.strict_bb_all_engine_barrier()
            wout3 = wout.rearrange("(k p) v -> p k v", p=P)
            wout_sb = p5w.tile([P, H // P, V], BF16, tag="wout")
            for k in range(H // P):
                nc.sync.dma_start(wout_sb[:, k, :], wout3[:, k, :])
            bout_sb = p5b.tile([P, V], F32, tag="bout")
            nc.sync.dma_start(bout_sb[:], bout[0][None, :].broadcast_to((P, V)))
            for n in range(NVC):
                for m in range(MT):
                    ps = ps5.tile([P, NV], F32, space="PSUM", tag="ps5")
                    lhs = hsT[:, :, ds(4 * m + 1, 4), :]
                    for k in range(H // P):
                        nc.tensor.matmul(
                            ps[:],
                            lhs[:, k].rearrange("p a b -> p (a b)"),
                            wout_sb[:, k, ds(n * NV, NV)],
                            start=(k == 0), stop=(k == H // P - 1))
                    osb = p5o.tile([P, NV], F32, tag="osb")
                    nc.vector.tensor_add(osb[:], ps[:], bout_sb[:, ds(n * NV, NV)])
                    nc.sync.dma_start(logits[n, m], osb[:])

    nc.finalize()
    return nc


_NC_CACHE: dict = {}


def _get_nc(debug: bool = False):
    key = bool(debug)
    if key not in _NC_CACHE:
        _NC_CACHE[key] = build_nc(debug=key)
    return _NC_CACHE[key]


def host_prep(inputs: dict) -> list[dict]:
    """Shard + lay out inputs for the 8 cores."""
    X = np.asarray(inputs["X"], dtype=np.float32)
    labels = np.asarray(inputs["labels"])
    W_f = np.asarray(inputs["W_f"], dtype=np.float32)
    b_f = np.asarray(inputs["b_f"], dtype=np.float32)
    emb = np.asarray(inputs["emb"], dtype=np.float32)
    W_ih = np.asarray(inputs["W_ih"], dtype=np.float32)
    W_hh = np.asarray(inputs["W_hh"], dtype=np.float32)
    b_ih = np.asarray(inputs["b_ih"], dtype=np.float32)
    b_hh = np.asarray(inputs["b_hh"], dtype=np.float32)
    W_out = np.asarray(inputs["W_out"], dtype=np.float32)
    b_out = np.asarray(inputs["b_out"], dtype=np.float32)

    bf16 = ml_dtypes.bfloat16
    perm = gate_perm()
    wft = np.ascontiguousarray(W_f.T).astype(bf16)                     # [IN, E]
    wihe = np.ascontiguousarray(W_ih[:, E:].T[:, perm]).astype(bf16)   # [E, 4H]
    wihf_aug = np.zeros((E + P, G4), dtype=np.float32)
    wihf_aug[:E] = W_ih[:, :E].T[:, perm]
    wihf_aug[E] = (b_ih + b_hh)[perm]
    wihf_aug = wihf_aug.astype(bf16)
    whh = np.ascontiguousarray(W_hh.T[:, perm]).astype(bf16)
    wout = np.ascontiguousarray(W_out.T).astype(bf16)                  # [H, V]

    shared = {
        "wft": wft, "bf": b_f[None, :], "embt": emb.astype(bf16), "wihe": wihe,
        "wihf": wihf_aug, "whh": whh, "wout": wout,
        "bout": b_out[None, :],
    }
    shifted = np.roll(labels, 1, axis=1)                               # [B, T]
    in_maps = []
    for c in range(NCORES):
        s = slice(c * BL, (c + 1) * BL)
        xt = np.ascontiguousarray(X[s].T).astype(bf16)                 # [IN, 32]
        lab = np.ascontiguousarray(shifted[s].T.reshape(TB, 1)).astype(np.int32)
        in_maps.append({**shared, "xt": xt, "lab": lab})
    return in_maps


def run(inputs: dict, debug: bool = False, trace: bool = False):
    nc = _get_nc(debug=debug)
    in_maps = host_prep(inputs)
    r = run_bass_kernel_spmd(nc, in_maps, core_ids=list(range(NCORES)), trace=trace)
    outs = []
    for c in range(NCORES):
        raw = r.results[c]["logits"].reshape(NVC, MT, 4, BL, NV)
        outs.append(np.ascontiguousarray(
            raw.transpose(3, 1, 2, 0, 4).reshape(BL, T, V)))
    out = np.concatenate(outs, axis=0)
    return out, r


def kernel(**inputs) -> np.ndarray:
    out, _ = run(inputs, debug=False, trace=False)
    return out


if __name__ == "__main__":
    rng = np.random.default_rng(0)
    fake = {
        "X": rng.standard_normal((B, IN)).astype(np.float32),
        "labels": rng.integers(0, V, size=(B, T)).astype(np.int64),
        "W_f": (rng.standard_normal((E, IN)) * 0.02).astype(np.float32),
        "b_f": np.zeros(E, np.float32),
        "emb": (rng.standard_normal((V + 1, E)) * 0.02).astype(np.float32),
        "W_ih": (rng.standard_normal((G4, 2 * E)) * 0.02).astype(np.float32),
        "W_hh": (rng.standard_normal((G4, H)) * 0.02).astype(np.float32),
        "b_ih": np.zeros(G4, np.float32),
        "b_hh": np.zeros(G4, np.float32),
        "W_out": (rng.standard_normal((V, H)) * 0.02).astype(np.float32),
        "b_out": np.zeros(V, np.float32),
    }
    out = kernel(**fake)
    print("out", out.shape, out.dtype, float(np.abs(out).max()))


# revision 9
# speedup vs baseline: 1.0683x; 1.0683x over previous
"""CCRNN Trainium2 kernel: feature MLP + embedding lookup + 40-step LSTM + vocab projection.

Sharding: data-parallel over batch B=256 -> 8 cores x 32. Weights replicated.

Per-core plan (B_loc=32, T=40, IN=2048, E=512, H=1024, V=5000):
  ph1: gather shifted embeddings (indirect DMA, bf16) + PE-transpose -> embT [E, tb]
  ph2: features = X @ W_f.T + b_f (bf16 matmul), augment with ones col, transpose,
       replicate 4x along M -> featT_rep [P, 5, 128] bf16
  ph3: gx[t,b,:] stored in SBUF as gxsb [128(4t',32b), MT, 8n, 512] bf16.
       Per (n, m): 4 emb matmuls into psum; vector add with fg_sb (feature+bias
       part, computed once per n on replicated featT) -> gxsb slice.
  ph4: LSTM recurrence, bf16 matmuls col-group packed (tile_position=(0,32q)).
       gt [128(q,b), 1024] prefetched per-step from gxsb via 8 SBUF->SBUF DMAs
       (partition shuffle (t',b)->(q,b)). h PE-transposed into hsT.
  ph5: logits = hs @ W_out.T + b_out (bf16, M=128 tiles), wout preloaded with
       contiguous 10KB/partition lines.

Gate column permutation (newcol -> orig), n = a*4+q in 0..7, j in 0..511:
  a=0: [i_q | f_q], a=1: [g_q | o_q], where gate rows (torch order) i,f,g,o
  and quarter q covers H rows 256q..256q+255.
"""

import os
import sys
import types
import contextlib

import numpy as np
import ml_dtypes

# ---- environment bootstrap (works in a bare dir; paths are machine-level) ----
for _p in ("/root/.axon_site", "/root/.axon_site/_ro/trn_rl_repo",
           "/root/.axon_site/_ro/pypackages"):
    if _p not in sys.path and os.path.isdir(_p):
        sys.path.insert(0, _p)

# shim the NTFF profile hook if the image's antenv lacks it (needed for trace=True)
try:
    from antenv.axon_hooks import get_axon_ntff_profile_hook  # noqa: F401
except ImportError:
    try:
        from trn_agent_boot.trn_boot import _ntff_profile_via_ctypes
        _m = types.ModuleType("antenv.axon_hooks")
        _hook = _ntff_profile_via_ctypes("/opt/axon/libaxon_pjrt.so")
        _m.get_axon_ntff_profile_hook = lambda: _hook
        _m.set_axon_ntff_profile_hook = lambda h: None
        import antenv  # noqa: F401
        sys.modules["antenv.axon_hooks"] = _m
    except Exception:
        pass

import concourse.bass as bass
import concourse.tile as tile
from concourse import mybir, bacc
from concourse import bass_utils
bass_utils.upload_artifacts = lambda tmpdir: f"file://{tmpdir}"
from concourse.bass import ds, ts
from concourse.bass_utils import run_bass_kernel_spmd
from concourse.masks import make_identity

P = 128
B, T = 256, 40
IN, E, H, V = 2048, 512, 1024, 5000
G4 = 4 * H
NCORES = 8
BL = B // NCORES            # 32 per core
TB = T * BL                 # 1280 (t*32+b) rows
MT = TB // P                # 10 M-tiles
F32 = mybir.dt.float32
F32R = mybir.dt.float32r
BF16 = mybir.dt.bfloat16
I32 = mybir.dt.int32

NV = 500                    # vocab chunk (psum free)
NVC = V // NV               # 10


def gate_perm() -> np.ndarray:
    """newcol -> orig index over the 4096 gate dim.

    bank0 (first psum bank) = [f | g] (the LSTM critical-path gates),
    bank1 = [i | o]; torch gate order is i,f,g,o.
    """
    perm = np.empty(G4, dtype=np.int64)
    for n in range(8):
        a, q = divmod(n, 4)
        for j in range(512):
            if a == 0:
                g = 1 if j < 256 else 2                # f | g
            else:
                g = 0 if j < 256 else 3                # i | o
            jj = j if j < 256 else j - 256
            perm[n * 512 + j] = g * H + q * 256 + jj
    return perm


def build_nc(debug: bool = False):
    nc = bacc.Bacc("TRN2", target_bir_lowering=False, debug=False)

    xt = nc.declare_dram_parameter("xt", [IN, BL], BF16, isOutput=False)
    lab = nc.declare_dram_parameter("lab", [TB, 1], I32, isOutput=False)
    wft = nc.declare_dram_parameter("wft", [IN, E], BF16, isOutput=False)
    bf = nc.declare_dram_parameter("bf", [1, E], F32, isOutput=False)
    embt = nc.declare_dram_parameter("embt", [V + 1, E], BF16, isOutput=False)
    wihe = nc.declare_dram_parameter("wihe", [E, G4], BF16, isOutput=False)
    wihf = nc.declare_dram_parameter("wihf", [E + P, G4], BF16, isOutput=False)
    whh = nc.declare_dram_parameter("whh", [H, G4], BF16, isOutput=False)
    wout = nc.declare_dram_parameter("wout", [H, V], BF16, isOutput=False)
    bout = nc.declare_dram_parameter("bout", [1, V], F32, isOutput=False)
    logits = nc.declare_dram_parameter("logits", [NVC, MT, P, NV], F32, isOutput=True)
    if debug:
        d_feat = nc.declare_dram_parameter("d_feat", [BL, E], F32, isOutput=True)
        d_gx0 = nc.declare_dram_parameter("d_gx0", [P, 1024], F32, isOutput=True)
        d_g0 = nc.declare_dram_parameter("d_g0", [P, 1024], F32, isOutput=True)
        d_h1 = nc.declare_dram_parameter("d_h1", [P, 256], F32, isOutput=True)
        d_c1 = nc.declare_dram_parameter("d_c1", [P, 256], F32, isOutput=True)
        d_g1 = nc.declare_dram_parameter("d_g1", [P, 1024], F32, isOutput=True)
        d_h2 = nc.declare_dram_parameter("d_h2", [P, 256], F32, isOutput=True)
        d_hsT = nc.declare_dram_parameter("d_hsT", [P, 8, BL], BF16, isOutput=True)

    Sig = mybir.ActivationFunctionType.Sigmoid
    Tanh = mybir.ActivationFunctionType.Tanh

    with tile.TileContext(nc) as tc, contextlib.ExitStack() as top:
        pc = top.enter_context(tc.tile_pool(name="pc", bufs=1))
        identb = pc.tile([P, P], BF16, tag="identb")
        make_identity(nc, identb[:])
        identf = pc.tile([P, P], F32, tag="identf")
        make_identity(nc, identf[:])

        # persistent across ph4+ph5
        prec = top.enter_context(tc.tile_pool(name="prec", bufs=1))
        hsT = prec.tile([P, 8, T + 1, BL], BF16, tag="hsT")
        cst = prec.tile([P, 256], F32, tag="cst")

        with tc.tile_pool(name="pgx", bufs=1) as pgx:
            # gx, bf16, natural ph3 layout: [128(4t',32b), m, n, 512]
            gxsb = pgx.tile([P, MT, 8, 512], BF16, tag="gxsb")

            with tc.tile_pool(name="pembT", bufs=1) as pembT, \
                 tc.tile_pool(name="pfeat", bufs=1) as pfeat:
                embT = pembT.tile([P, E // P, TB], BF16, tag="embT")
                featT_rep = pfeat.tile([P, 5, P], BF16, tag="featT_rep")

                # ---------------- phase 1: gather + transpose embeddings ----------------
                with (
                    tc.tile_pool(name="pgath", bufs=1) as pgath,
                    tc.tile_pool(name="ps1", bufs=4, space="PSUM") as ps1,
                ):
                    lab_sb = pgath.tile([P, MT, 1], I32, tag="lab")
                    nc.sync.dma_start(lab_sb[:], lab.rearrange("(m p) o -> p m o", p=P))
                    gath = pgath.tile([P, MT, E], BF16, tag="gath")
                    for m in range(MT):
                        nc.gpsimd.indirect_dma_start(
                            out=gath[:, m, :], out_offset=None, in_=embt[:],
                            in_offset=bass.IndirectOffsetOnAxis(ap=lab_sb[:, m, :], axis=0),
                        )
                    for m in range(MT):
                        for k in range(E // P):
                            pst = ps1.tile([P, P], BF16, space="PSUM", tag="pst")
                            nc.tensor.transpose(pst[:], gath[:, m, ds(k * P, P)], identb[:])
                            nc.vector.tensor_copy(embT[:, k, ds(m * P, P)], pst[:])

                # ---------------- phase 2: features ----------------
                with (
                    tc.tile_pool(name="pph2", bufs=1) as p2,
                    tc.tile_pool(name="ps2", bufs=2, space="PSUM") as ps2,
                    tc.tile_pool(name="ps2f", bufs=1, space="PSUM") as ps2f,
                ):
                    xt_sb = p2.tile([P, IN // P, BL], BF16, tag="xt")
                    nc.sync.dma_start(xt_sb[:], xt.rearrange("(k p) b -> p k b", p=P))
                    wft_sb = p2.tile([P, IN // P, E], BF16, tag="wft")
                    nc.sync.dma_start(wft_sb[:], wft.rearrange("(k p) e -> p k e", p=P))
                    bf_sb = p2.tile([BL, E], F32, tag="bf")
                    nc.sync.dma_start(bf_sb[:], bf[0][None, :].broadcast_to((BL, E)))

                    psf = ps2f.tile([BL, E], F32, space="PSUM", tag="psf")
                    nk = IN // P
                    for k in range(nk):
                        nc.tensor.matmul(psf[:], xt_sb[:, k, :], wft_sb[:, k, :],
                                         start=(k == 0), stop=(k == nk - 1))
                    fp = p2.tile([P, E + P], F32, tag="fp")
                    nc.gpsimd.memset(fp[:], 0.0)
                    nc.vector.tensor_add(fp[:BL, 0:E], psf[:], bf_sb[:])
                    nc.gpsimd.memset(fp[:BL, E:E + 1], 1.0)
                    if debug:
                        nc.sync.dma_start(d_feat[:], fp[:BL, 0:E])

                    for kk in range(5):
                        pst2 = ps2.tile([P, P], F32, space="PSUM", tag="pst2")
                        nc.tensor.transpose(pst2[:], fp[:, ds(kk * P, P)], identf[:])
                        for r in range(4):
                            nc.vector.tensor_copy(featT_rep[:, kk, ds(r * BL, BL)],
                                                  pst2[:, 0:BL])

                # ---------------- phase 3: gx ----------------
                with (
                    tc.tile_pool(name="p3w", bufs=2) as p3w,
                    tc.tile_pool(name="ps3", bufs=3, space="PSUM") as ps3,
                    tc.tile_pool(name="ps3f", bufs=2, space="PSUM") as ps3f,
                ):
                    wihe3 = wihe.rearrange("(k p) c -> p k c", p=P)
                    wihf3 = wihf.rearrange("(k p) c -> p k c", p=P)
                    for n in range(8):
                        wihe_n = p3w.tile([P, E // P, 512], BF16, tag="wihe")
                        nc.sync.dma_start(wihe_n[:], wihe3[:, :, ds(n * 512, 512)])
                        wihf_n = p3w.tile([P, 5, 512], BF16, tag="wihf")
                        nc.sync.dma_start(wihf_n[:], wihf3[:, :, ds(n * 512, 512)])

                        psfg = ps3f.tile([P, 512], F32, space="PSUM", tag="psfg")
                        for kk in range(5):
                            nc.tensor.matmul(psfg[:], featT_rep[:, kk, :], wihf_n[:, kk, :],
                                             start=(kk == 0), stop=(kk == 4))
                        fg_sb = p3w.tile([P, 512], BF16, tag="fg")
                        nc.scalar.copy(fg_sb[:], psfg[:])

                        for m in range(MT):
                            ps = ps3.tile([P, 512], F32, space="PSUM", tag="ps3")
                            for k in range(E // P):
                                nc.tensor.matmul(ps[:], embT[:, k, ds(m * P, P)],
                                                 wihe_n[:, k, :],
                                                 start=(k == 0), stop=(k == E // P - 1))
                            nc.vector.tensor_add(gxsb[:, m, n, :], ps[:], fg_sb[:])

            # ---------------- phase 4: recurrence ----------------
            with (
                tc.tile_pool(name="pwhh", bufs=1) as pwhh,
                tc.tile_pool(name="p4", bufs=2) as p4,
                tc.tile_pool(name="p4g", bufs=1) as p4g,
                tc.tile_pool(name="ps4", bufs=3, space="PSUM") as ps4,
                tc.tile_pool(name="ps4t", bufs=2, space="PSUM") as ps4t,
            ):
                whh_sb = pwhh.tile([P, H // P, G4], BF16, tag="whh")
                whh3 = whh.rearrange("(k p) c -> p k c", p=P)
                for k in range(H // P):
                    nc.sync.dma_start(whh_sb[:, k, :], whh3[:, k, :])
                nc.gpsimd.memset(hsT[:, :, 0, :], 0.0)
                nc.gpsimd.memset(cst[:], 0.0)
                NGT = 3
                gxt = [p4g.tile([P, 1024], BF16, tag=f"gxt{i}", name=f"gxt{i}")
                       for i in range(NGT)]
                tc.strict_bb_all_engine_barrier()

                def prefetch_gt(t):
                    # gt[(q,b), a*512+j] <- gxsb[(t%4, b), t//4, a*4+q, j]
                    gt = gxt[t % NGT]
                    tp, m = t % 4, t // 4
                    for n8 in range(8):
                        a8, q8 = divmod(n8, 4)
                        nc.sync.dma_start(
                            gt[ds(32 * q8, 32), ds(512 * a8, 512)],
                            gxsb[ds(32 * tp, 32), m, n8, :])
                    return gt

                gts = {}
                for t in range(2):
                    gts[t] = prefetch_gt(t)

                hsT_v = hsT[:].rearrange("p (q h) t b -> p q h t b", h=2)

                for t in range(T):
                    if t + 2 < T:
                        gts[t + 2] = prefetch_gt(t + 2)
                    gt = gts.pop(t)
                    pss = []
                    for a in range(2):
                        ps = ps4.tile([P, 512], F32, space="PSUM", tag="ps4")
                        pss.append(ps)
                        for k in range(H // P):
                            for q in range(4):
                                nc.tensor.matmul(
                                    ps[ds(32 * q, 32), :],
                                    hsT[:, k, t, :],
                                    whh_sb[:, k, ds((a * 4 + q) * 512, 512)],
                                    start=(k == 0), stop=(k == H // P - 1),
                                    tile_position=(0, 32 * q))
                    # bank0 = [f | g], bank1 = [i | o]
                    ga = p4.tile([P, 512], F32, tag="ga")
                    gb = p4.tile([P, 512], F32, tag="gb")
                    nc.vector.tensor_add(ga[:, 0:256], pss[0][:, 0:256], gt[:, 0:256])
                    nc.vector.tensor_add(ga[:, 256:512], pss[0][:, 256:512],
                                         gt[:, 256:512])
                    sf = p4.tile([P, 256], F32, tag="sf")
                    tg = p4.tile([P, 256], F32, tag="tg")
                    nc.scalar.activation(sf[:], ga[:, 0:256], Sig)
                    nc.scalar.activation(tg[:], ga[:, 256:512], Tanh)
                    nc.vector.tensor_mul(cst[:], sf[:], cst[:])
                    nc.vector.tensor_add(gb[:, 0:256], pss[1][:, 0:256], gt[:, 512:768])
                    si = p4.tile([P, 256], F32, tag="si")
                    nc.scalar.activation(si[:], gb[:, 0:256], Sig)
                    nc.vector.tensor_add(gb[:, 256:512], pss[1][:, 256:512],
                                         gt[:, 768:1024])
                    so = p4.tile([P, 256], F32, tag="so")
                    nc.scalar.activation(so[:], gb[:, 256:512], Sig)
                    # transpose so while the c-chain finishes
                    soT = []
                    for half in range(2):
                        pso = ps4t.tile([P, P], F32, space="PSUM", tag="pso")
                        nc.tensor.transpose(pso[:], so[:, ds(half * P, P)], identf[:])
                        soT.append(pso)
                    t1 = p4.tile([P, 256], F32, tag="t1")
                    nc.vector.tensor_mul(t1[:], si[:], tg[:])
                    nc.vector.tensor_add(cst[:], cst[:], t1[:])
                    if debug and t == 0:
                        nc.sync.dma_start(d_g0[:, 0:512], ga[:])
                        nc.sync.dma_start(d_g0[:, 512:1024], gb[:])
                        nc.sync.dma_start(d_gx0[:, 0:512], gt[:, 0:512])
                        nc.sync.dma_start(d_gx0[:, 512:1024], gt[:, 512:1024])

                    # tail: transpose c, tanh on transposed halves, multiply with
                    # soT writing hsT directly
                    for half in range(2):
                        pct = ps4t.tile([P, P], F32, space="PSUM", tag="pct")
                        nc.tensor.transpose(pct[:], cst[:, ds(half * P, P)], identf[:])
                        tcT = p4.tile([P, P], F32, tag="tcT")
                        nc.scalar.activation(tcT[:], pct[:], Tanh)
                        nc.vector.tensor_mul(
                            hsT_v[:, :, half, t + 1, :],
                            soT[half][:].rearrange("p (q b) -> p q b", b=BL),
                            tcT[:].rearrange("p (q b) -> p q b", b=BL))
                    if debug and t in (0, 1):
                        h128 = p4.tile([P, 256], F32, tag="h128")
                        tcf = p4.tile([P, 256], F32, tag="tcf")
                        nc.scalar.activation(tcf[:], cst[:], Tanh)
                        nc.vector.tensor_mul(h128[:], so[:], tcf[:])
                        if t == 0:
                            nc.sync.dma_start(d_h1[:], h128[:])
                            nc.sync.dma_start(d_c1[:], cst[:])
                        if t == 1:
                            nc.sync.dma_start(d_hsT[:], hsT[:, :, 1, :])
                            nc.sync.dma_start(d_g1[:, 0:512], ga[:])
                            nc.sync.dma_start(d_g1[:, 512:1024], gb[:])
                            nc.sync.dma_start(d_h2[:], h128[:])

        # ---------------- phase 5: output projection ----------------
        with (
            tc.tile_pool(name="p5w", bufs=1) as p5w,
            tc.tile_pool(name="p5o", bufs=3) as p5o,
            tc.tile_pool(name="p5b", bufs=1) as p5b,
            tc.tile_pool(name="ps5", bufs=4, space="PSUM") as ps5,
        ):
            tc.strict_bb_all_engine_barrier()
            wout3 = wout.rearrange("(k p) v -> p k v", p=P)
            wout_sb = p5w.tile([P, H // P, V], BF16, tag="wout")
            for k in range(H // P):
                nc.sync.dma_start(wout_sb[:, k, :], wout3[:, k, :])
            bout_sb = p5b.tile([P, V], F32, tag="bout")
            nc.sync.dma_start(bout_sb[:], bout[0][None, :].broadcast_to((P, V)))
            for n in range(NVC):
                for m in range(MT):
                    ps = ps5.tile([P, NV], F32, space="PSUM", tag="ps5")
                    lhs = hsT[:, :, ds(4 * m + 1, 4), :]
                    for k in range(H // P):
                        nc.tensor.matmul(
                            ps[:],
                            lhs[:, k].rearrange("p a b -> p (a b)"),
                            wout_sb[:, k, ds(n * NV, NV)],
                            start=(k == 0), stop=(k == H // P - 1))
                    osb = p5o.tile([P, NV], F32, tag="osb")
                    nc.vector.tensor_add(osb[:], ps[:], bout_sb[:, ds(n * NV, NV)])
                    nc.sync.dma_start(logits[n, m], osb[:])

    nc.finalize()
    return nc


_NC_CACHE: dict = {}


def _get_nc(debug: bool = False):
    key = bool(debug)
    if key not in _NC_CACHE:
        _NC_CACHE[key] = build_nc(debug=key)
    return _NC_CACHE[key]


def host_prep(inputs: dict) -> list[dict]:
    """Shard + lay out inputs for the 8 cores."""
    X = np.asarray(inputs["X"], dtype=np.float32)
    labels = np.asarray(inputs["labels"])
    W_f = np.asarray(inputs["W_f"], dtype=np.float32)
    b_f = np.asarray(inputs["b_f"], dtype=np.float32)
    emb = np.asarray(inputs["emb"], dtype=np.float32)
    W_ih = np.asarray(inputs["W_ih"], dtype=np.float32)
    W_hh = np.asarray(inputs["W_hh"], dtype=np.float32)
    b_ih = np.asarray(inputs["b_ih"], dtype=np.float32)
    b_hh = np.asarray(inputs["b_hh"], dtype=np.float32)
    W_out = np.asarray(inputs["W_out"], dtype=np.float32)
    b_out = np.asarray(inputs["b_out"], dtype=np.float32)

    bf16 = ml_dtypes.bfloat16
    perm = gate_perm()
    wft = np.ascontiguousarray(W_f.T).astype(bf16)                     # [IN, E]
    wihe = np.ascontiguousarray(W_ih[:, E:].T[:, perm]).astype(bf16)   # [E, 4H]
    wihf_aug = np.zeros((E + P, G4), dtype=np.float32)
    wihf_aug[:E] = W_ih[:, :E].T[:, perm]
    wihf_aug[E] = (b_ih + b_hh)[perm]
    wihf_aug = wihf_aug.astype(bf16)
    whh = np.ascontiguousarray(W_hh.T[:, perm]).astype(bf16)
    wout = np.ascontiguousarray(W_out.T).astype(bf16)                  # [H, V]

    shared = {
        "wft": wft, "bf": b_f[None, :], "embt": emb.astype(bf16), "wihe": wihe,
        "wihf": wihf_aug, "whh": whh, "wout": wout,
        "bout": b_out[None, :],
    }
    shifted = np.roll(labels, 1, axis=1)                               # [B, T]
    in_maps = []
    for c in range(NCORES):
        s = slice(c * BL, (c + 1) * BL)
        xt = np.ascontiguousarray(X[s].T).astype(bf16)                 # [IN, 32]
        lab = np.ascontiguousarray(shifted[s].T.reshape(TB, 1)).astype(np.int32)
        in_maps.append({**shared, "xt": xt, "lab": lab})
    return in_maps


def run(inputs: dict, debug: bool = False, trace: bool = False):
    nc = _get_nc(debug=debug)
    in_maps = host_prep(inputs)
    r = run_bass_kernel_spmd(nc, in_maps, core_ids=list(range(NCORES)), trace=trace)
    outs = []
    for c in range(NCORES):
        raw = r.results[c]["logits"].reshape(NVC, MT, 4, BL, NV)
        outs.append(np.ascontiguousarray(
            raw.transpose(3, 1, 2, 0, 4).reshape(BL, T, V)))
    out = np.concatenate(outs, axis=0)
    return out, r


def kernel(**inputs) -> np.ndarray:
    out, _ = run(inputs, debug=False, trace=False)
    return out


if __name__ == "__main__":
    rng = np.random.default_rng(0)
    fake = {
        "X": rng.standard_normal((B, IN)).astype(np.float32),
        "labels": rng.integers(0, V, size=(B, T)).astype(np.int64),
        "W_f": (rng.standard_normal((E, IN)) * 0.02).astype(np.float32),
        "b_f": np.zeros(E, np.float32),
        "emb": (rng.standard_normal((V + 1, E)) * 0.02).astype(np.float32),
        "W_ih": (rng.standard_normal((G4, 2 * E)) * 0.02).astype(np.float32),
        "W_hh": (rng.standard_normal((G4, H)) * 0.02).astype(np.float32),
        "b_ih": np.zeros(G4, np.float32),
        "b_hh": np.zeros(G4, np.float32),
        "W_out": (rng.standard_normal((V, H)) * 0.02).astype(np.float32),
        "b_out": np.zeros(V, np.float32),
    }
    out = kernel(**fake)
    print("out", out.shape, out.dtype, float(np.abs(out).max()))
